# revision 8
# baseline (speedup 1.0000x reference)
"""Trainium2 Bass kernel for nn_HGT (2-layer Heterogeneous Graph Transformer).

Self-contained: host preprocessing (edge sort/shard, weight folding) + SPMD
Bass/Tile kernel on 8 NeuronCores + output assembly.

Strategy (destination-node partitioned, edge-parallel):
- Pad node counts to multiples of 128; 128-node "blocks" are the unit of work.
  sample: 16 blocks, mrna: 160, mirna: 16  ->  each core owns 2/20/2 blocks.
- Per layer, every core redundantly computes the small dense projections
  (K/V "tables" with relation transforms, per-head p_rel/scale folded into
  the weights) for ALL nodes; Q only for its own nodes.
- Edges are sorted by destination on host, padded per block to a uniform
  chunk count, and processed 128 edges/chunk: indirect-DMA gather of k/v and
  q rows, per-head dot-product scores on DVE, exp on ACT, and a one-hot
  matmul on the PE that segment-sums numerator (exp*v) and denominator (exp)
  into a PSUM accumulator per destination block.  No segment-max is needed:
  |score| <= ~6 for this model, exp() is safe in f32.
- Finalize per block: agg = N/(D+eps), exact gelu via Erf, output projection
  with the skip connection folded in (beta into Wa/ba, (1-beta) via a scaled
  identity matmul), all in a transposed [feat, node] layout so biases are
  per-partition.
- One AllGather (the only collective) exchanges x^(1) blocks between layers.
"""
import sys
import os
import time

sys.path.insert(0, "/opt/trn_rl_repo")
import numpy as np

import concourse.bass as bass
import concourse.mybir as mybir
import concourse.tile as tile
import concourse.bacc as bacc
import concourse.bass_utils as bass_utils
from concourse.masks import make_identity

P = 128
HID, HEADS, D, OUT = 128, 4, 32, 32
NT = ['sample', 'mrna', 'mirna']
NPAD = {'sample': 2048, 'mrna': 20480, 'mirna': 2048}
NREAL = {'sample': 2000, 'mrna': 20000, 'mirna': 2000}
IN_DIMS = {'sample': 512, 'mrna': 256, 'mirna': 128}
EDGE_META = {'s2m': ('sample', 'mrna'), 'm2s': ('mrna', 'sample'),
             's2i': ('sample', 'mirna'), 'i2s': ('mirna', 'sample')}
DST_GROUPS = {'sample': ['m2s', 'i2s'], 'mrna': ['s2m'], 'mirna': ['s2i']}
G_CH = {'sample': 8, 'mrna': 6, 'mirna': 8}     # chunks per gather group
OWN = {'sample': 2, 'mrna': 20, 'mirna': 2}     # blocks per core per type
TYPE_BASE = {'sample': 0, 'mrna': 2, 'mirna': 22}
NCORES = 8
NOWN = 24                                        # own blocks per core
F32 = mybir.dt.float32
I32 = mybir.dt.int32


# ------------------------------------------------------------------ host prep

def _sigmoid(x):
    return 1.0 / (1.0 + np.exp(-x))


def prep_weights(params):
    scale = 1.0 / np.sqrt(D)
    layers = []
    for p in params['convs']:
        L = {'wkv': {}, 'bkv': {}, 'wq': {}, 'bq': {}, 'wab': {}, 'ba': {}, 'imb': {}}
        for et, (s_nt, d_nt) in EDGE_META.items():
            A = np.zeros((HID, HID), np.float32)
            M = np.zeros((HID, HID), np.float32)
            for h in range(HEADS):
                A[h*D:(h+1)*D, h*D:(h+1)*D] = np.asarray(p['a_rel'][et][h])
                M[h*D:(h+1)*D, h*D:(h+1)*D] = np.asarray(p['m_rel'][et][h])
            Wk = np.asarray(p['Wk'][s_nt]) @ A
            bk = np.asarray(p['bk'][s_nt]) @ A
            for h in range(HEADS):
                f = float(p['p_rel'][et][h]) * scale
                Wk[:, h*D:(h+1)*D] *= f
                bk[h*D:(h+1)*D] *= f
            Wv = np.asarray(p['Wv'][s_nt]) @ M
            bv = np.asarray(p['bv'][s_nt]) @ M
            L['wkv'][et] = np.ascontiguousarray(np.concatenate([Wk, Wv], 1), np.float32)
            L['bkv'][et] = np.concatenate([bk, bv]).reshape(1, 256).astype(np.float32)
        for nt in NT:
            L['wq'][nt] = np.ascontiguousarray(np.asarray(p['Wq'][nt]), np.float32)
            L['bq'][nt] = np.asarray(p['bq'][nt]).reshape(1, HID).astype(np.float32)
            beta = float(_sigmoid(np.asarray(p['skip'][nt])))
            L['wab'][nt] = np.ascontiguousarray(0.5 * beta * np.asarray(p['Wa'][nt]), np.float32)
            L['ba'][nt] = (beta * np.asarray(p['ba'][nt])).reshape(HID, 1).astype(np.float32)
            L['imb'][nt] = ((1.0 - beta) * np.eye(HID)).astype(np.float32)
        layers.append(L)
    return layers


def prep_edges(inputs):
    """Sort by destination, pad to uniform per-block chunk counts, shard."""
    edges = {'s2m': inputs['ei_s2m'], 'm2s': inputs['ei_m2s'],
             's2i': inputs['ei_s2i'], 'i2s': inputs['ei_i2s']}
    out = {}
    for t in NT:
        srcs, dsts = [], []
        off = 0
        offsets = {}
        for et in DST_GROUPS[t]:
            s_nt = EDGE_META[et][0]
            offsets[et] = off
            srcs.append(np.asarray(edges[et][0]).astype(np.int64) + off)
            dsts.append(np.asarray(edges[et][1]).astype(np.int64))
            off += NPAD[s_nt]
        srcu = np.concatenate(srcs)
        dst = np.concatenate(dsts)
        order = np.argsort(dst, kind='stable')
        srcu, dst = srcu[order], dst[order]
        nblk = NPAD[t] // P
        cnt = np.bincount(dst // P, minlength=nblk)
        G = G_CH[t]
        C = int(-(-int(cnt.max()) // P))
        C = int(-(-C // G) * G)
        slots = C * P
        starts = np.zeros(nblk + 1, np.int64)
        starts[1:] = np.cumsum(cnt)
        ngrp = OWN[t] * C // G
        meta_i = np.zeros((NCORES, ngrp, P, 2, G), np.int32)
        meta_f = np.zeros((NCORES, ngrp, P, 2, G), np.float32)
        dloc = dst % P
        for gb in range(nblk):
            c = gb // OWN[t]
            pos_t = gb % OWN[t]
            own_pos = TYPE_BASE[t] + pos_t
            n_b = int(cnt[gb])
            su = np.zeros(slots, np.int32)
            dl = np.zeros(slots, np.int32)
            mk = np.zeros(slots, np.float32)
            su[:n_b] = srcu[starts[gb]:starts[gb+1]]
            dl[:n_b] = dloc[starts[gb]:starts[gb+1]]
            mk[:n_b] = 1.0
            qi = own_pos * P + dl
            g0 = pos_t * (C // G)
            meta_i[c, g0:g0 + C//G, :, 0, :] = su.reshape(C // G, P, G)
            meta_i[c, g0:g0 + C//G, :, 1, :] = qi.reshape(C // G, P, G)
            meta_f[c, g0:g0 + C//G, :, 0, :] = dl.reshape(C // G, P, G).astype(np.float32)
            meta_f[c, g0:g0 + C//G, :, 1, :] = mk.reshape(C // G, P, G)
        out[t] = dict(meta_i=meta_i, meta_f=meta_f, C=C, G=G, ngrp=ngrp,
                      offsets=offsets, nblk=nblk)
    return out


def own_blocks(c):
    res = []
    for t in NT:
        for i in range(OWN[t]):
            res.append((TYPE_BASE[t] + i, t, c * OWN[t] + i))
    return res


# ------------------------------------------------------------- device program

def build_nc(cfg):
    """Build + compile the SPMD Bass program. cfg: {t: (C, G, ngrp)}."""
    nc = bacc.Bacc("TRN2", target_bir_lowering=False, debug=False,
                   num_devices=NCORES)

    def din(name, shape, dtype=F32):
        return nc.dram_tensor(name, shape, dtype, kind="ExternalInput").ap()

    xin = {t: din(f"xin_{t}", [IN_DIMS[t], NPAD[t]]) for t in NT}
    wlin = {t: din(f"wlin_{t}", [IN_DIMS[t], HID]) for t in NT}
    blin = {t: din(f"blin_{t}", [HID, 1]) for t in NT}
    wkv = {(l, et): din(f"wkv{l}_{et}", [HID, 256]) for l in (0, 1) for et in EDGE_META}
    bkv = {(l, et): din(f"bkv{l}_{et}", [1, 256]) for l in (0, 1) for et in EDGE_META}
    wq = {(l, t): din(f"wq{l}_{t}", [HID, HID]) for l in (0, 1) for t in NT}
    bq = {(l, t): din(f"bq{l}_{t}", [1, HID]) for l in (0, 1) for t in NT}
    wab = {(l, t): din(f"wab{l}_{t}", [HID, HID]) for l in (0, 1) for t in NT}
    ba = {(l, t): din(f"ba{l}_{t}", [HID, 1]) for l in (0, 1) for t in NT}
    imb = {(l, t): din(f"imb{l}_{t}", [HID, HID]) for l in (0, 1) for t in NT}
    wout_d = din("wout", [HID, OUT])
    bout_d = din("bout", [1, OUT])
    meta_i = {t: din(f"meta_i_{t}", [cfg[t][2], P, 2, cfg[t][1]], I32) for t in NT}
    meta_f = {t: din(f"meta_f_{t}", [cfg[t][2], P, 2, cfg[t][1]]) for t in NT}
    ownidx = {t: din(f"ownidx_{t}", [P, OWN[t]], I32) for t in NT}

    xout = nc.dram_tensor("xout", [NOWN, P, HID], F32, kind="ExternalOutput").ap()
    logits_out = nc.dram_tensor("logits_out", [2 * P, OUT], F32, kind="ExternalOutput").ap()

    TAB_ROWS = {t: sum(NPAD[EDGE_META[et][0]] for et in DST_GROUPS[t]) for t in NT}

    from contextlib import ExitStack
    with tile.TileContext(nc) as tc, ExitStack() as stack:
        cw = stack.enter_context(tc.tile_pool(name="consts", bufs=1))
        dram = stack.enter_context(tc.tile_pool(name="dram", bufs=1, space="DRAM"))
        io = stack.enter_context(tc.tile_pool(name="io", bufs=3))
        eg = stack.enter_context(tc.tile_pool(name="edge", bufs=3))
        fz = stack.enter_context(tc.tile_pool(name="fin", bufs=2))
        ps_mm = stack.enter_context(tc.tile_pool(name="psmm", bufs=2, space="PSUM"))
        ps_ag = stack.enter_context(tc.tile_pool(name="psag", bufs=3, space="PSUM"))
        ps_tr = stack.enter_context(tc.tile_pool(name="pstr", bufs=2, space="PSUM"))

        # ---------------- DRAM internals
        x0T = {t: dram.tile([NPAD[t] // P, P, P], F32, name=f"x0T_{t}") for t in NT}
        kvtab = {}
        qtab = {}
        for l in (0, 1):
            for t in NT:
                kvtab[(l, t)] = dram.tile([TAB_ROWS[t], 256], F32, name=f"kv{l}_{t}")
            qtab[l] = dram.tile([NOWN * P, HID], F32, name=f"q{l}")
        ag_in = dram.tile([NOWN, P, P], F32, name="ag_in")
        ag_out = dram.tile([NCORES * NOWN, P, P], F32, name="ag_out",
                           addr_space="Shared")

        # ---------------- constants to SBUF
        iota_t = cw.tile([P, P], F32, name="iota")
        nc.gpsimd.iota(iota_t[:], pattern=[[1, P]], base=0, channel_multiplier=0,
                       allow_small_or_imprecise_dtypes=True)
        ident = cw.tile([P, P], F32, name="ident")
        make_identity(nc, ident[:])
        ones1 = cw.tile([1, P], F32, name="ones1")
        nc.vector.memset(ones1[:], 1.0)

        def load_const(name, ap_, shape, dtype=F32):
            t_ = cw.tile(shape, dtype, name=name)
            nc.sync.dma_start(out=t_[:], in_=ap_[:])
            return t_

        # wlin sample is [512,128] in DRAM; load as 4 SBUF slices of [128,128]
        wlin_sl = {}
        for t in NT:
            ks = IN_DIMS[t] // P
            tl = cw.tile([P, ks * HID], F32, name=f"wlinS_{t}")
            for k in range(ks):
                nc.sync.dma_start(out=tl[:, k*HID:(k+1)*HID],
                                  in_=wlin[t][k*P:(k+1)*P, :])
            wlin_sl[t] = tl
        blin_s = {t: load_const(f"blin_{t}", blin[t], [HID, 1]) for t in NT}
        wkv_s = {k: load_const(f"wkv_{k}", v, [HID, 256]) for k, v in wkv.items()}
        bkv_s = {k: load_const(f"bkv_{k}", v, [1, 256]) for k, v in bkv.items()}
        wq_s = {k: load_const(f"wq_{k}", v, [HID, HID]) for k, v in wq.items()}
        bq_s = {k: load_const(f"bq_{k}", v, [1, HID]) for k, v in bq.items()}
        wab_s = {k: load_const(f"wab_{k}", v, [HID, HID]) for k, v in wab.items()}
        ba_s = {k: load_const(f"ba_{k}", v, [HID, 1]) for k, v in ba.items()}
        imb_s = {k: load_const(f"imb_{k}", v, [HID, HID]) for k, v in imb.items()}
        wout_s = load_const("wout", wout_d, [HID, OUT])
        bout_s = load_const("bout", bout_d, [1, OUT])

        own_x = {l: cw.tile([P, NOWN * P], F32, name=f"own_x{l}") for l in (0, 1)}

        # ---------------- LIN phase: x0 = lrelu(x_in @ Wlin + b), all blocks
        # transposed layout: psum [f_out, n]; 2 node-blocks per matmul
        NB2 = 2  # node blocks per iteration
        for t in NT:
            nblk = NPAD[t] // P
            ks = IN_DIMS[t] // P
            for b0 in range(0, nblk, NB2):
                xr = io.tile([P, ks, NB2 * P], F32, name="lin_xr")
                for k in range(ks):
                    nc.sync.dma_start(out=xr[:, k, :],
                                      in_=xin[t][k*P:(k+1)*P, b0*P:(b0+NB2)*P])
                ps = ps_mm.tile([P, NB2 * P], F32, name="mm_ps")
                for k in range(ks):
                    nc.tensor.matmul(ps[:], lhsT=wlin_sl[t][:, k*HID:(k+1)*HID],
                                     rhs=xr[:, k, :],
                                     start=(k == 0), stop=(k == ks - 1))
                xb = io.tile([P, NB2 * P], F32, name="lin_xb")
                # bias (per-partition) then lrelu in one DVE op each
                nc.scalar.activation(xb[:], ps[:], mybir.ActivationFunctionType.Identity,
                                     bias=blin_s[t][:, 0:1])
                nc.vector.scalar_tensor_tensor(
                    out=xb[:], in0=xb[:], scalar=0.01, in1=xb[:],
                    op0=mybir.AluOpType.mult, op1=mybir.AluOpType.max)
                for j in range(NB2):
                    nc.sync.dma_start(out=x0T[t][b0 + j], in_=xb[:, j*P:(j+1)*P])

        # own-block extraction for layer 0 (uniform across cores via data indices)
        for t in NT:
            oi = io.tile([P, OWN[t]], I32, name="own_oi")
            nc.sync.dma_start(out=oi[:], in_=ownidx[t][:, :])
            for i in range(OWN[t]):
                pos = TYPE_BASE[t] + i
                nc.gpsimd.indirect_dma_start(
                    out=own_x[0][:, pos*P:(pos+1)*P],
                    out_offset=None,
                    in_=x0T[t][:].rearrange("b p q -> (b p) q"),
                    in_offset=bass.IndirectOffsetOnAxis(ap=oi[:, i:i+1], axis=0),
                )

        # ---------------- per-layer helpers
        def build_tables(l):
            """kv tables (all nodes, redundant) + q table (own nodes)."""
            # q first (own blocks; unblocks edge phases earliest)
            for pos, t, _gb in own_blocks(0):  # gb unused: own_x is position-based
                xb_sl = own_x[l][:, pos*P:(pos+1)*P]
                ps = ps_mm.tile([P, 256], F32, name="mm_ps")
                nc.tensor.matmul(ps[:, 0:HID], lhsT=xb_sl, rhs=wq_s[(l, t)][:],
                                 start=True, stop=False)
                nc.tensor.matmul(ps[:, 0:HID], lhsT=ones1[:], rhs=bq_s[(l, t)][:],
                                 start=False, stop=True)
                qb = io.tile([P, HID], F32, name="q_qb")
                nc.scalar.activation(qb[:], ps[:, 0:HID], mybir.ActivationFunctionType.Copy)
                nc.sync.dma_start(out=qtab[l][pos*P:(pos+1)*P, :], in_=qb[:])

            # kv tables: order mrna-dst, mirna-dst (small, unblock edge phases),
            # then sample-dst (large)
            for t in ('mrna', 'mirna', 'sample'):
                for et in DST_GROUPS[t]:
                    s_nt = EDGE_META[et][0]
                    roff = None
                    off = 0
                    for et2 in DST_GROUPS[t]:
                        if et2 == et:
                            roff = off
                        off += NPAD[EDGE_META[et2][0]]
                    for b in range(NPAD[s_nt] // P):
                        xb = io.tile([P, P], F32, name="tab_xb")
                        if l == 0:
                            nc.sync.dma_start(out=xb[:], in_=x0T[s_nt][b])
                        else:
                            r, pos = b // OWN[s_nt], b % OWN[s_nt]
                            flat = r * NOWN + TYPE_BASE[s_nt] + pos
                            nc.sync.dma_start(out=xb[:], in_=ag_out[flat])
                        ps = ps_mm.tile([P, 256], F32, name="mm_ps")
                        nc.tensor.matmul(ps[:], lhsT=xb[:], rhs=wkv_s[(l, et)][:],
                                         start=True, stop=False)
                        nc.tensor.matmul(ps[:], lhsT=ones1[:], rhs=bkv_s[(l, et)][:],
                                         start=False, stop=True)
                        tb = io.tile([P, 256], F32, name="tab_tb")
                        nc.scalar.activation(tb[:], ps[:],
                                             mybir.ActivationFunctionType.Copy)
                        nc.sync.dma_start(
                            out=kvtab[(l, t)][roff + b*P: roff + (b+1)*P, :],
                            in_=tb[:])

        def edge_phase(l):
            """For each own dst block: gather, score, exp, one-hot matmul."""
            for t in ('mrna', 'mirna', 'sample'):
                C, G, ngrp = cfg[t]
                gpb = C // G  # groups per block
                for pos_t in range(OWN[t]):
                    pos = TYPE_BASE[t] + pos_t
                    psb = ps_ag.tile([P, 132], F32, name="agg_ps")
                    for g in range(gpb):
                        gi = pos_t * gpb + g
                        mi = eg.tile([P, 2 * G], I32, name="eg_mi")
                        nc.sync.dma_start(out=mi[:],
                                          in_=meta_i[t][gi].rearrange("p a g -> p (a g)"))
                        mf = eg.tile([P, 2 * G], F32, name="eg_mf")
                        nc.sync.dma_start(out=mf[:],
                                          in_=meta_f[t][gi].rearrange("p a g -> p (a g)"))
                        kv_t = eg.tile([P, G, 256], F32, name="eg_kv")
                        for j in range(G):
                            nc.gpsimd.indirect_dma_start(
                                out=kv_t[:, j, :], out_offset=None,
                                in_=kvtab[(l, t)][:],
                                in_offset=bass.IndirectOffsetOnAxis(
                                    ap=mi[:, j:j+1], axis=0))
                        qe = eg.tile([P, G, HID], F32, name="eg_qe")
                        for j in range(G):
                            nc.gpsimd.indirect_dma_start(
                                out=qe[:, j, :], out_offset=None,
                                in_=qtab[l][:],
                                in_offset=bass.IndirectOffsetOnAxis(
                                    ap=mi[:, G+j:G+j+1], axis=0))
                        onehot = eg.tile([P, G, P], F32, name="eg_oh")
                        nc.vector.tensor_tensor(
                            out=onehot[:],
                            in0=iota_t[:, None, :].to_broadcast([P, G, P]),
                            in1=mf[:, 0:G][:, :, None].to_broadcast([P, G, P]),
                            op=mybir.AluOpType.is_equal)
                        qk = eg.tile([P, G, HID], F32, name="eg_qk")
                        nc.vector.tensor_tensor(out=qk[:], in0=qe[:],
                                                in1=kv_t[:, :, 0:HID],
                                                op=mybir.AluOpType.mult)
                        sc = eg.tile([P, G, HEADS], F32, name="eg_sc")
                        nc.vector.tensor_reduce(
                            out=sc[:],
                            in_=qk[:].rearrange("p g (h d) -> p g h d", h=HEADS),
                            axis=mybir.AxisListType.X, op=mybir.AluOpType.add)
                        ex = eg.tile([P, G, HEADS], F32, name="eg_ex")
                        nc.scalar.activation(ex[:], sc[:],
                                             mybir.ActivationFunctionType.Exp)
                        rhs = eg.tile([P, G, 132], F32, name="eg_rhs")
                        # masked exp -> rhs[:, :, 128:132]
                        nc.vector.tensor_tensor(
                            out=rhs[:, :, HID:132], in0=ex[:],
                            in1=mf[:, G:2*G][:, :, None].to_broadcast([P, G, HEADS]),
                            op=mybir.AluOpType.mult)
                        # v * exp -> rhs[:, :, 0:128]
                        nc.vector.tensor_tensor(
                            out=rhs[:, :, 0:HID].rearrange("p g (h d) -> p g h d", h=HEADS),
                            in0=kv_t[:, :, HID:256].rearrange("p g (h d) -> p g h d", h=HEADS),
                            in1=rhs[:, :, HID:132][:, :, :, None].to_broadcast(
                                [P, G, HEADS, D]),
                            op=mybir.AluOpType.mult)
                        for j in range(G):
                            nc.tensor.matmul(psb[:], lhsT=onehot[:, j, :],
                                             rhs=rhs[:, j, :],
                                             start=(g == 0 and j == 0),
                                             stop=(g == gpb - 1 and j == G - 1))
                    finalize(l, t, pos, psb)

        def finalize(l, t, pos, psb):
            dn = fz.tile([P, HEADS], F32, name="fin_dn")
            nc.scalar.activation(dn[:], psb[:, HID:132],
                                 mybir.ActivationFunctionType.Copy, bias=1e-16)
            rc = fz.tile([P, HEADS], F32, name="fin_rc")
            nc.vector.reciprocal(rc[:], dn[:])
            agg = fz.tile([P, HID], F32, name="fin_agg")
            nc.vector.tensor_tensor(
                out=agg[:].rearrange("p (h d) -> p h d", h=HEADS),
                in0=psb[:, 0:HID].rearrange("p (h d) -> p h d", h=HEADS),
                in1=rc[:][:, :, None].to_broadcast([P, HEADS, D]),
                op=mybir.AluOpType.mult)
            er = fz.tile([P, HID], F32, name="fin_er")
            nc.scalar.activation(er[:], agg[:], mybir.ActivationFunctionType.Erf,
                                 scale=float(1.0 / np.sqrt(2.0)))
            g2 = fz.tile([P, HID], F32, name="fin_g2")
            # g2 = (erf + 1) * agg  == 2*gelu(agg); the 0.5 is folded into wab
            nc.vector.scalar_tensor_tensor(
                out=g2[:], in0=er[:], scalar=1.0, in1=agg[:],
                op0=mybir.AluOpType.add, op1=mybir.AluOpType.mult)
            pst = ps_tr.tile([P, P], F32, name="tr_ps")
            nc.tensor.transpose(out=pst[:], in_=g2[:], identity=ident[:])
            g2t = fz.tile([P, P], F32, name="fin_g2t")
            nc.scalar.activation(g2t[:], pst[:], mybir.ActivationFunctionType.Copy)
            ps2 = ps_tr.tile([P, P], F32, name="tr_ps")
            nc.tensor.matmul(ps2[:], lhsT=wab_s[(l, t)][:], rhs=g2t[:],
                             start=True, stop=False)
            nc.tensor.matmul(ps2[:], lhsT=imb_s[(l, t)][:],
                             rhs=own_x[l][:, pos*P:(pos+1)*P],
                             start=False, stop=True)
            if l == 0:
                xnew = own_x[1][:, pos*P:(pos+1)*P]
                nc.scalar.activation(xnew, ps2[:],
                                     mybir.ActivationFunctionType.Identity,
                                     bias=ba_s[(l, t)][:, 0:1])
                nc.sync.dma_start(out=ag_in[pos], in_=xnew)
            else:
                xnew = fz.tile([P, P], F32, name="fin_xnew")
                nc.scalar.activation(xnew[:], ps2[:],
                                     mybir.ActivationFunctionType.Identity,
                                     bias=ba_s[(l, t)][:, 0:1])
                nc.sync.dma_start(out=xout[pos], in_=xnew[:])
                if t == 'sample':
                    # logits for own sample blocks
                    ps3 = ps_mm.tile([P, 256], F32, name="mm_ps")
                    nc.tensor.matmul(ps3[:, 0:OUT], lhsT=xnew[:], rhs=wout_s[:],
                                     start=True, stop=False)
                    nc.tensor.matmul(ps3[:, 0:OUT], lhsT=ones1[:], rhs=bout_s[:],
                                     start=False, stop=True)
                    lg = fz.tile([P, OUT], F32, name="log_lg")
                    nc.scalar.activation(lg[:], ps3[:, 0:OUT],
                                         mybir.ActivationFunctionType.Copy)
                    nc.sync.dma_start(out=logits_out[pos*P:(pos+1)*P, :], in_=lg[:])

        # ---------------- layer 0
        build_tables(0)
        edge_phase(0)
        # ---------------- allgather x1 blocks
        nc.gpsimd.collective_compute(
            "AllGather", mybir.AluOpType.bypass,
            replica_groups=[list(range(NCORES))],
            ins=[ag_in[:].rearrange("b p q -> (b p) q")],
            outs=[ag_out[:].rearrange("b p q -> (b p) q")],
        )
        # ---------------- layer 1
        build_tables(1)
        edge_phase(1)

    nc.compile()
    return nc


_NC_CACHE = {}


def _get_nc(cfg_key):
    if cfg_key not in _NC_CACHE:
        cfg = {t: cfg_key[i] for i, t in enumerate(NT)}
        _NC_CACHE[cfg_key] = build_nc(cfg)
    return _NC_CACHE[cfg_key]


# ------------------------------------------------------------------ top level

def _pad_xT(x, t):
    xp = np.zeros((NPAD[t], IN_DIMS[t]), np.float32)
    xp[:x.shape[0]] = np.asarray(x, np.float32)
    return np.ascontiguousarray(xp.T)


def make_inmaps(inputs):
    params = inputs['params']
    layers = prep_weights(params)
    ED = prep_edges(inputs)

    shared = {}
    for t in NT:
        shared[f"xin_{t}"] = _pad_xT(inputs['x_' + t], t)
        shared[f"wlin_{t}"] = np.ascontiguousarray(np.asarray(params['lin'][t]['W'], np.float32))
        shared[f"blin_{t}"] = np.asarray(params['lin'][t]['b'], np.float32).reshape(HID, 1)
    for l, L in enumerate(layers):
        for et in EDGE_META:
            shared[f"wkv{l}_{et}"] = L['wkv'][et]
            shared[f"bkv{l}_{et}"] = L['bkv'][et]
        for t in NT:
            shared[f"wq{l}_{t}"] = L['wq'][t]
            shared[f"bq{l}_{t}"] = L['bq'][t]
            shared[f"wab{l}_{t}"] = L['wab'][t]
            shared[f"ba{l}_{t}"] = L['ba'][t]
            shared[f"imb{l}_{t}"] = L['imb'][t]
    shared["wout"] = np.ascontiguousarray(np.asarray(params['out']['W'], np.float32))
    shared["bout"] = np.asarray(params['out']['b'], np.float32).reshape(1, OUT)

    in_maps = []
    for c in range(NCORES):
        m = dict(shared)
        for t in NT:
            m[f"meta_i_{t}"] = np.ascontiguousarray(ED[t]['meta_i'][c])
            m[f"meta_f_{t}"] = np.ascontiguousarray(ED[t]['meta_f'][c])
            oi = np.zeros((P, OWN[t]), np.int32)
            for i in range(OWN[t]):
                oi[:, i] = (c * OWN[t] + i) * P + np.arange(P)
            m[f"ownidx_{t}"] = oi
        in_maps.append(m)
    cfg_key = tuple((ED[t]['C'], ED[t]['G'], ED[t]['ngrp']) for t in NT)
    return in_maps, cfg_key


def assemble(results):
    x_full = {t: np.zeros((NPAD[t], HID), np.float32) for t in NT}
    logits = np.zeros((NPAD['sample'], OUT), np.float32)
    for c in range(NCORES):
        xo = results[c]["xout"]
        lo = results[c]["logits_out"]
        for pos, t, gb in own_blocks(c):
            x_full[t][gb*P:(gb+1)*P, :] = xo[pos].T
            if t == 'sample':
                pos_t = pos - TYPE_BASE['sample']
                logits[gb*P:(gb+1)*P, :] = lo[pos_t*P:(pos_t+1)*P, :]
    x = {t: x_full[t][:NREAL[t]] for t in NT}
    return x, logits[:NREAL['sample']]


def kernel(x_sample, x_mrna, x_mirna, ei_s2m, ei_m2s, ei_s2i, ei_i2s, params):
    inputs = dict(x_sample=x_sample, x_mrna=x_mrna, x_mirna=x_mirna,
                  ei_s2m=ei_s2m, ei_m2s=ei_m2s, ei_s2i=ei_s2i, ei_i2s=ei_i2s,
                  params=params)
    in_maps, cfg_key = make_inmaps(inputs)
    nc = _get_nc(cfg_key)
    res = bass_utils.run_bass_kernel_spmd(nc, in_maps, core_ids=list(range(NCORES)))
    return assemble(res.results)
